# revision 1
# baseline (speedup 1.0000x reference)
"""Trainium2 Bass kernel for nn_DeformableAttention (B=4, C=384, H=W=56, NH=12, HC=32, STRIDE=2).

Self-contained: hardcodes shapes/sharding. Sharding: 8 cores = 4 batches x 2
pixel-row-halves. Each core computes the full value/key/offset branches for its
batch (duplicated across the pair) and the query branch + final GEMM for its
half of the 3136 output pixels.

Math note: the reference computes out = (scale * q^T k) v^T without softmax, so
attention is linear and reassociates:
    y[b] = (w_out @ blockdiag_h(scale * M[b,h])) @ Q[b],
    M[b,h] = V_s[b,h] K[b,h]^T  (32x32 per head)
which drops the 48x(3136x784x32) einsums to a few small GEMMs.

v3 layout notes:
  - x is padded to 58x58 and convs run on the flat 58-wide raster, so every
    stride-1 tap is a contiguous 2D slice (output cols x=56,57 are garbage and
    never read; pixel index = y*58+x).
  - value/key convs run on the PE as diag-weight matmuls over fp8(e4m3, x16
    pre-scale) inputs using DoubleRow perf mode: taps are paired (0,3),(1,4),
    (2,5),(6,7)+8 and each pair's two shifted x views ride the 2-k-tile dim of
    one matmul. PSUM accumulates in f32; the PSUM->SBUF copy divides by 16.
  - value chunks are PE-transposed into a pixel-major fp16 vtab in DRAM; the 4
    bilinear neighbors are fetched with 4 dma_gather ops (wrap-16 int16 index
    layout built via one-hot-column matmuls), giving point-major [128, 7, 384]
    tiles directly.
  - the offset branch runs in fp16 on DVE (fp16 scalars so the 2x element
    mode can engage), with LN statistics via fp16 PE matmuls into f32 PSUM.
  - conv PSUM pools are entered up front so transient off-branch pools never
    overlap their space (overlap would serialize the conv stream behind the
    whole offset branch).
"""
import contextlib

import numpy as np
import ml_dtypes

import concourse.bass as bass
import concourse.tile as tile
from concourse import bacc, mybir
from concourse.bass_utils import run_bass_kernel_spmd

F32, F16, I32, I16 = mybir.dt.float32, mybir.dt.float16, mybir.dt.int32, mybir.dt.int16
F8 = mybir.dt.float8e4
DR = mybir.MatmulPerfMode.DoubleRow
MULT, ADD, SUB = mybir.AluOpType.mult, mybir.AluOpType.add, mybir.AluOpType.subtract
MIN = mybir.AluOpType.min
AF = mybir.ActivationFunctionType

B, C, H, W = 4, 384, 56, 56
NH, HC = 12, 32
SCALE = HC ** -0.5
FSC = 16.0                      # fp8 weight pre-scale (undone in PSUM copy)
HP = H + 2                      # 58: padded row width; flat pixel = y*58+x
XW = HP * HP + 2                # 3366: padded x cols (+2 so last tap stays in-bounds)
PIXF = H * HP                   # 3248 flat conv-output pixels (y in [0,56))
KH = KW = 28                    # stride-2 output
N = KH * KW                     # 784 offset points
NCH = 7                         # point chunks of 128 (last has 16 valid)
HALF_ROWS = H // 2              # 28
QF = HALF_ROWS * HP             # 1624 flat query cols per core
QXW = (HALF_ROWS + 2) * HP + 2  # 1742: per-core query input (30 rows + tap pad)
HALF_PIX = HALF_ROWS * W        # 1568 valid output pixels per core
PIXH = HALF_ROWS * HP           # 1624 flat pixels in this core's half
CT = C // 128                   # 3 channel tiles
EPS = 1e-5
IMAX = float(H - 2)             # 54: floor clamp so idx+59 stays valid

# tap pairing for DoubleRow: (base flat offset, delta to partner)
PAIRS = ((0, 58), (1, 58), (2, 58), (116, 1))   # taps (0,3) (1,4) (2,5) (6,7)
SING = 118                                      # tap 8
CTSPAN = 4 * 256 + 128                          # 1152 dv8 cols per (conv, ct)

_CACHE = {}


def _emit(nc, tc, ctx, io):
    pool = ctx.enter_context(tc.tile_pool(name="main", bufs=1))
    dma = nc.sync

    # ---------------- loads ----------------
    xh = []
    for ct in range(CT):
        t = pool.tile([128, XW], F16, tag=f"xh_{ct}")
        dma.dma_start(t[:], io["xh"][ct * 128:(ct + 1) * 128, :])
        xh.append(t)
    dv16 = pool.tile([128, 60 * 128], F16, tag="dv16")
    dma.dma_start(dv16[:], io["dv16"][:, :])
    wp = []
    for ct in range(CT):
        t = pool.tile([128, 32], F32, tag=f"wp_{ct}")
        dma.dma_start(t[:], io["wpack"][ct * 128:(ct + 1) * 128, :])
        wp.append(t)
    # wpack cols: 0-8 wq, 18-26 wo, 27 bq, 29 bo, 30 lng, 31 lnb
    xq = []
    for ct in range(CT):
        t = pool.tile([128, QXW], F16, tag=f"xq_{ct}")
        dma.dma_start(t[:], io["xq"][ct * 128:(ct + 1) * 128, :])
        xq.append(t)
    w2 = []
    for ct in range(CT):
        t = pool.tile([128, 2], F32, tag=f"w2_{ct}")
        dma.dma_start(t[:], io["w2t"][ct * 128:(ct + 1) * 128, :])
        w2.append(t)
    wot16 = []
    for ct in range(CT):
        t = pool.tile([128, C], F16, tag=f"wot_{ct}")
        dma.dma_start(t[:], io["wot"][ct * 128:(ct + 1) * 128, :])
        wot16.append(t)
    refyx = pool.tile([2, N], F32, tag="refyx")
    dma.dma_start(refyx[:], io["refyx"][:, :])
    identF = pool.tile([128, 128], F32, tag="identF")
    dma.dma_start(identF[:], io["identF"][:, :])
    ident16 = pool.tile([128, 128], F16, tag="ident16")
    dma.dma_start(ident16[:], io["ident16"][:, :])
    one_row = pool.tile([1, 128], F32, tag="one_row")
    nc.vector.memset(one_row[:], 1.0)
    ones_col = pool.tile([128, 1], F32, tag="ones_col")
    nc.vector.memset(ones_col[:], 1.0)
    ones16c = pool.tile([128, 512], F16, tag="ones16c")
    nc.vector.memset(ones16c[:], 1.0)

    # long-lived conv PSUM pools (see module docstring)
    vcp = tc.tile_pool(name="vc_ps", bufs=2, space="PSUM")
    vcpool = vcp.__enter__()
    vtp = tc.tile_pool(name="vt_ps", bufs=2, space="PSUM")
    vtpool = vtp.__enter__()

    def diag_conv(ps_col, blk0, xt, rhs_off, rhs_tail):
        """9-tap fp16 diag-matmul conv accumulation into ps_col (no stop)."""
        xa = xt[:]
        for tp in range(9):
            dy, dx = tp // 3, tp % 3
            rhs = bass.AP(xa.tensor, xa.offset + dy * HP + dx + rhs_off,
                          [[xa.ap[0][0], 128]] + rhs_tail)
            nc.tensor.matmul(ps_col, dv16[:, (blk0 + tp) * 128:(blk0 + tp + 1) * 128],
                             rhs, start=(tp == 0), stop=False)

    # ---------------- value conv (PE fp16 diag matmuls) ----------------
    val = []
    CHUNKS = [(c * 512, min(512, PIXF - c * 512)) for c in range((PIXF + 511) // 512)]
    for ct in range(CT):
        t = pool.tile([128, PIXF], F16, tag=f"val_{ct}")
        for c0, cw in CHUNKS:
            ps = vcpool.tile([128, 512], F32, tag="vc", space="PSUM")
            diag_conv(ps[:, :cw], ct * 9, xh[ct], c0, [[1, cw]])
            nc.tensor.matmul(ps[:, :cw], dv16[:, (54 + ct) * 128:(55 + ct) * 128],
                             ones16c[:, :cw], start=False, stop=True)
            nc.scalar.activation(t[:, c0:c0 + cw], ps[:, :cw], AF.Copy)
        val.append(t)

    # ---------------- key conv (PE fp16 diag, stride-2 views; emitted late so value conv owns PE early) ----------------
    key = []
    KCH = ((0, 16), (16, 12))   # row-chunks of the 28x28 output
    for ct in range(CT):
        t = pool.tile([128, N], F16, tag=f"key_{ct}")
        for r0, rows in KCH:
            cw = rows * KW
            ps = vcpool.tile([128, 512], F32, tag="vc", space="PSUM")
            diag_conv(ps[:, :cw], 27 + ct * 9, xh[ct], 2 * r0 * HP,
                      [[2 * HP, rows], [2, KW]])
            nc.tensor.matmul(ps[:, :cw], dv16[:, (57 + ct) * 128:(58 + ct) * 128],
                             ones16c[:, :cw], start=False, stop=True)
            nc.scalar.activation(t[:, r0 * KW:r0 * KW + cw], ps[:, :cw], AF.Copy)
        key.append(t)

    # ---------------- vtab transposes (PE) + DRAM writes ----------------
    vtab_writes = []
    PCH = [(c * 128, min(128, PIXF - c * 128)) for c in range((PIXF + 127) // 128)]
    with tc.tile_pool(name="vt_sb", bufs=3) as vts:
        for p0, pw in PCH:
            ps = vtpool.tile([128, C], F16, tag="vt", space="PSUM")
            for ct in range(CT):
                nc.tensor.transpose(ps[:pw, ct * 128:(ct + 1) * 128],
                                    val[ct][:, p0:p0 + pw], ident16[:, :])
            sb = vts.tile([128, C], F16, tag="vt_sb")
            nc.scalar.activation(sb[:pw, :], ps[:pw, :], AF.Copy)
            wi = dma.dma_start(io["vtab"][p0:p0 + pw, :], sb[:pw, :])
            vtab_writes.append(wi)

    # ---------------- off conv (DVE fp16) + LayerNorm + GELU ----------------
    off = []
    for ct in range(CT):
        t = pool.tile([128, N], F32, tag=f"off_{ct}")
        o2 = t[:].rearrange("p (h w) -> p h w", h=KH)
        x3 = xh[ct][:, :HP * HP].rearrange("p (h w) -> p h w", h=HP)
        for tp in range(9):
            dy, dx = tp // 3, tp % 3
            src = x3[:, dy:dy + 2 * KH - 1:2, dx:dx + 2 * KW - 1:2]
            if tp == 0:
                nc.vector.tensor_scalar(out=o2, in0=src, scalar1=wp[ct][:, 18:19],
                                        scalar2=wp[ct][:, 29:30], op0=MULT, op1=ADD)
            else:
                nc.vector.scalar_tensor_tensor(out=o2, in0=src,
                                               scalar=wp[ct][:, 18 + tp:19 + tp],
                                               in1=o2, op0=MULT, op1=ADD)
        off.append(t)

    sq = []
    for ct in range(CT):
        t = pool.tile([128, N], F32, tag=f"sq_{ct}")
        nc.scalar.activation(t[:], off[ct][:], AF.Square)
        sq.append(t)

    SLICES = (slice(0, 512), slice(512, N))
    mu = pool.tile([1, N], F32, tag="mu_sb")
    es = pool.tile([1, N], F32, tag="es_sb")
    with tc.tile_pool(name="ln_ps", bufs=1, space="PSUM") as lnp:
        mu_ps = lnp.tile([1, N], F32, tag="mu_ps")
        ssq_ps = lnp.tile([1, N], F32, tag="ssq_ps")
        for sl in SLICES:
            for ct in range(CT):
                nc.tensor.matmul(mu_ps[:, sl], ones_col[:, :], off[ct][:, sl],
                                 start=(ct == 0), stop=(ct == CT - 1))
            for ct in range(CT):
                nc.tensor.matmul(ssq_ps[:, sl], ones_col[:, :], sq[ct][:, sl],
                                 start=(ct == 0), stop=(ct == CT - 1))
        nc.scalar.activation(mu[:], mu_ps[:], AF.Copy, scale=1.0 / C)
        nc.scalar.activation(es[:], ssq_ps[:], AF.Copy, scale=1.0 / C)
    musq = pool.tile([1, N], F32, tag="musq")
    nc.scalar.activation(musq[:], mu[:], AF.Square)
    var = pool.tile([1, N], F32, tag="var")
    nc.vector.tensor_tensor(out=var[:], in0=es[:], in1=musq[:], op=SUB)
    nc.vector.tensor_scalar_add(var[:], var[:], EPS)
    sd = pool.tile([1, N], F32, tag="sd")
    nc.scalar.activation(sd[:], var[:], AF.Sqrt)
    rstd = pool.tile([1, N], F32, tag="rstd")
    nc.vector.reciprocal(rstd[:], sd[:])
    mu_b = pool.tile([128, N], F32, tag="mu_b")
    rstd_b = pool.tile([128, N], F32, tag="rstd_b")
    with tc.tile_pool(name="bc_ps", bufs=2, space="PSUM") as bcp:
        for src, dst in ((mu, mu_b), (rstd, rstd_b)):
            for sl in SLICES:
                w_ = sl.stop - sl.start
                bc_ps = bcp.tile([128, 512], F32, tag="bc", space="PSUM")
                nc.tensor.matmul(bc_ps[:, :w_], one_row[:, :], src[:, sl],
                                 start=True, stop=True)
                nc.scalar.activation(dst[:, sl], bc_ps[:, :w_], AF.Copy)

    gel = []
    for ct in range(CT):
        t1 = sq[ct]
        nc.vector.tensor_tensor(out=t1[:], in0=off[ct][:], in1=mu_b[:], op=SUB)
        nc.vector.tensor_tensor(out=t1[:], in0=t1[:], in1=rstd_b[:], op=MULT)
        nc.vector.tensor_scalar(out=t1[:], in0=t1[:], scalar1=wp[ct][:, 30:31],
                                scalar2=wp[ct][:, 31:32], op0=MULT, op1=ADD)
        g = off[ct]
        nc.scalar.activation(g[:], t1[:], AF.Gelu)
        gel.append(g)

    # ---------------- offsets -> point-major iy/ix [128, (7,2)] ----------------
    pos = pool.tile([2, N], F32, tag="pos")
    with tc.tile_pool(name="oyx_ps", bufs=1, space="PSUM") as oxp:
        o_ps = oxp.tile([2, N], F32, tag="o_ps")
        for sl in SLICES:
            for ct in range(CT):
                nc.tensor.matmul(o_ps[:, sl], w2[ct][:, :], gel[ct][:, sl],
                                 start=(ct == 0), stop=(ct == CT - 1))
        nc.vector.tensor_tensor(out=pos[:], in0=o_ps[:], in1=refyx[:], op=ADD)

    iyx = pool.tile([128, 14], F32, tag="iyx")
    with tc.tile_pool(name="iyx_ps", bufs=1, space="PSUM") as ixp:
        i_ps = ixp.tile([128, 14], F32, tag="i_ps")
        for k in range(NCH):
            kn = min(128, N - k * 128)
            nc.tensor.transpose(i_ps[:kn, 2 * k:2 * k + 2],
                                pos[:, k * 128:k * 128 + kn], identF[:2, :2])
        nc.scalar.activation(iyx[:], i_ps[:], AF.Tanh)
    # iy/ix = (tanh+1)*(H-1)/2
    nc.vector.tensor_scalar(out=iyx[:], in0=iyx[:], scalar1=(H - 1) / 2.0,
                            scalar2=(H - 1) / 2.0, op0=MULT, op1=ADD)

    # floor + clamp (exact floor whether the int cast truncates or rounds)
    xy0i = pool.tile([128, 14], I32, tag="xy0i")
    nc.vector.tensor_copy(xy0i[:], iyx[:])
    xy0f = pool.tile([128, 14], F32, tag="xy0f")
    nc.vector.tensor_copy(xy0f[:], xy0i[:])
    gtm = pool.tile([128, 14], F32, tag="gtm")
    nc.vector.tensor_tensor(out=gtm[:], in0=xy0f[:], in1=iyx[:], op=mybir.AluOpType.is_gt)
    nc.vector.tensor_tensor(out=xy0f[:], in0=xy0f[:], in1=gtm[:], op=SUB)
    nc.vector.tensor_scalar(out=xy0f[:], in0=xy0f[:], scalar1=IMAX, scalar2=None, op0=MIN)
    frac = pool.tile([128, 14], F32, tag="frac")
    nc.vector.tensor_tensor(out=frac[:], in0=iyx[:], in1=xy0f[:], op=SUB)
    omf = pool.tile([128, 14], F32, tag="omf")
    nc.vector.tensor_scalar(out=omf[:], in0=frac[:], scalar1=-1.0, scalar2=1.0,
                            op0=MULT, op1=ADD)

    # bilinear weights, point-major [128, (4j, 7k)]
    wts = pool.tile([128, 28], F32, tag="wts")
    f3 = frac[:].rearrange("p (k t) -> p k t", t=2)
    o3 = omf[:].rearrange("p (k t) -> p k t", t=2)
    nc.vector.tensor_tensor(out=wts[:, 0:7], in0=o3[:, :, 0], in1=o3[:, :, 1], op=MULT)
    nc.vector.tensor_tensor(out=wts[:, 7:14], in0=o3[:, :, 0], in1=f3[:, :, 1], op=MULT)
    nc.vector.tensor_tensor(out=wts[:, 14:21], in0=f3[:, :, 0], in1=o3[:, :, 1], op=MULT)
    nc.vector.tensor_tensor(out=wts[:, 21:28], in0=f3[:, :, 0], in1=f3[:, :, 1], op=MULT)
    # (pad points >= 784 only exist in chunk 6 partitions >= 16, which no
    # consumer reads: bilinear and M slice [:kn] there)

    # 4 gather index variants, point-major f32
    idx4 = pool.tile([128, 28], F32, tag="idx4")
    x3v = xy0f[:].rearrange("p (k t) -> p k t", t=2)
    nc.vector.scalar_tensor_tensor(out=idx4[:, 0:7], in0=x3v[:, :, 0], scalar=float(HP),
                                   in1=x3v[:, :, 1], op0=MULT, op1=ADD)
    for j, d in ((1, 1.0), (2, float(HP)), (3, float(HP + 1))):
        nc.vector.tensor_scalar(out=idx4[:, j * 7:j * 7 + 7], in0=idx4[:, 0:7],
                                scalar1=d, scalar2=None, op0=ADD)

    # wrap-16 int16 index layout for dma_gather: idxw[q, j, 8k+a] = idx_j of
    # point 128k+16a+q, built via one-hot-column matmuls + permuted-copy cast.
    idxw = pool.tile([128, 224], I16, tag="idxw")
    with tc.tile_pool(name="wr_ps", bufs=1, space="PSUM") as wrp:
        w_ps = wrp.tile([16, 224], F32, tag="w_ps")
        for a in range(8):
            nc.tensor.matmul(w_ps[:, a * 28:(a + 1) * 28],
                             identF[:, 16 * a:16 * (a + 1)], idx4[:, :],
                             start=True, stop=True)
        src = w_ps[:].rearrange("p (a v k) -> p v k a", a=8, v=4)
        dst = idxw[0:16, :].rearrange("p (v k a) -> p v k a", v=4, k=7)
        nc.vector.tensor_copy(dst, src)
    idxw_dmas = []
    for lo, n_ in ((16, 16), (32, 32), (64, 64)):
        di = dma.dma_start(idxw[lo:lo + n_, :], idxw[0:n_, :])
        idxw_dmas.append(di)

    # conv PSUM pools done (vtab transposes emitted above)
    vtp.__exit__(None, None, None)
    vcp.__exit__(None, None, None)

    # ---------------- gathers (dma_gather, 4 neighbors) ----------------
    g = []
    for j in range(4):
        t = pool.tile([128, NCH * C], F16, tag=f"g_{j}")
        gi = nc.gpsimd.dma_gather(
            out_ap=t[:].rearrange("p (k c) -> p k c", k=NCH),
            in_ap=io["vtab"][:, :],
            idxs_ap=idxw[:, j * 56:j * 56 + 49],
            num_idxs=N, num_idxs_reg=N, elem_size=C, queue_num=j)
        for wi in vtab_writes:
            tile.add_dep_helper(gi.ins, wi.ins, reason="vtab RAW")
        g.append(t)

    # bilinear combine, point-major: vs[p, k, c]. Act does the per-partition
    # scale mults (scale APs are legal there); DVE does fp16 TT adds (2x mode)
    vs = pool.tile([128, NCH * C], F16, tag="vs")
    tmp_b = pool.tile([128, NCH * C], F16, tag="tmp_b")
    for k in range(NCH):
        kn = min(128, N - k * 128)
        sl = slice(k * C, (k + 1) * C)
        nc.scalar.activation(vs[:kn, sl], g[0][:kn, sl], AF.Copy,
                             scale=wts[:kn, k:k + 1])
        nc.scalar.activation(tmp_b[:kn, sl], g[1][:kn, sl], AF.Copy,
                             scale=wts[:kn, 7 + k:8 + k])
        nc.vector.tensor_tensor(out=vs[:kn, sl], in0=vs[:kn, sl],
                                in1=tmp_b[:kn, sl], op=ADD)
        nc.scalar.activation(tmp_b[:kn, sl], g[2][:kn, sl], AF.Copy,
                             scale=wts[:kn, 14 + k:15 + k])
        nc.vector.tensor_tensor(out=vs[:kn, sl], in0=vs[:kn, sl],
                                in1=tmp_b[:kn, sl], op=ADD)
        nc.scalar.activation(tmp_b[:kn, sl], g[3][:kn, sl], AF.Copy,
                             scale=wts[:kn, 21 + k:22 + k])
        nc.vector.tensor_tensor(out=vs[:kn, sl], in0=vs[:kn, sl],
                                in1=tmp_b[:kn, sl], op=ADD)

    # ---------------- key transpose + ksum ----------------
    kT = []
    with tc.tile_pool(name="kt_ps", bufs=2, space="PSUM") as ktp:
        for k in range(NCH):
            kn = min(128, N - k * 128)
            ps = ktp.tile([128, C], F16, tag="kt_ps", space="PSUM")
            for ct in range(CT):
                nc.tensor.transpose(ps[:kn, ct * 128:(ct + 1) * 128],
                                    key[ct][:, k * 128:k * 128 + kn], ident16[:, :])
            t = pool.tile([128, C], F16, tag=f"kT_{k}")
            nc.scalar.activation(t[:kn, :], ps[:kn, :], AF.Copy)
            kT.append(t)

    # ---------------- M (per-ct block-diag heads) ----------------
    m16 = []
    with tc.tile_pool(name="m_ps", bufs=1, space="PSUM") as mps:
        for ct in range(CT):
            m_ps = mps.tile([128, 128], F32, tag=f"m_ps{ct}", name=f"m_ps{ct}")
            vsv = vs[:].rearrange("p (k c) -> p k c", k=NCH)
            for k in range(NCH):
                kn = min(128, N - k * 128)
                nc.tensor.matmul(m_ps[:, :], vsv[:kn, k, ct * 128:(ct + 1) * 128],
                                 kT[k][:kn, ct * 128:(ct + 1) * 128],
                                 start=(k == 0), stop=(k == NCH - 1))
            t = pool.tile([128, 128], F16, tag=f"m16_{ct}")
            nc.scalar.activation(t[:], m_ps[:], AF.Copy, scale=SCALE)
            m16.append(t)

    # ---------------- AT = blockdiag(M)^T W_out^T ----------------
    at16 = []
    with tc.tile_pool(name="at_ps", bufs=1, space="PSUM") as atp:
        for ct in range(CT):
            at_ps = atp.tile([128, C], F32, tag=f"at_ps{ct}", name=f"at_ps{ct}")
            for j in range(4):
                sl = slice(j * 32, (j + 1) * 32)
                nc.tensor.matmul(at_ps[sl, :], m16[ct][sl, sl], wot16[ct][sl, :],
                                 start=True, stop=True, tile_position=(j * 32, j * 32))
            t = pool.tile([128, C], F16, tag=f"at16_{ct}")
            nc.scalar.activation(t[:], at_ps[:], AF.Copy)
            at16.append(t)

    # ---------------- query conv (DVE fp16, flat) ----------------
    q16 = []
    for ct in range(CT):
        t = pool.tile([128, QF], F16, tag=f"q_{ct}")
        for tp in range(9):
            dy, dx = tp // 3, tp % 3
            src = xq[ct][:, dy * HP + dx:dy * HP + dx + QF]
            if tp == 0:
                nc.vector.tensor_scalar(out=t[:], in0=src, scalar1=wp[ct][:, 0:1],
                                        scalar2=wp[ct][:, 27:28], op0=MULT, op1=ADD)
            else:
                nc.vector.scalar_tensor_tensor(out=t[:], in0=src,
                                               scalar=wp[ct][:, tp:tp + 1],
                                               in1=t[:], op0=MULT, op1=ADD)
        q16.append(t)

    # ---------------- y = AT^T @ Q ----------------
    RPC = 7                     # image rows per output chunk
    CW = RPC * W                # 392
    with tc.tile_pool(name="y_ps", bufs=2, space="PSUM") as yps, \
         tc.tile_pool(name="y_sb", bufs=3) as ysb:
        for ot in range(CT):
            for pc in range(HALF_ROWS // RPC):
                y_ps = yps.tile([128, CW], F32, tag="y_ps", space="PSUM")
                for ct in range(CT):
                    qv = q16[ct][:].rearrange("p (h w) -> p h w", h=HALF_ROWS)
                    nc.tensor.matmul(y_ps[:], at16[ct][:, ot * 128:(ot + 1) * 128],
                                     qv[:, pc * RPC:(pc + 1) * RPC, 0:W],
                                     start=(ct == 0), stop=(ct == CT - 1))
                y_sb = ysb.tile([128, CW], F32, tag="y_sb")
                nc.scalar.activation(y_sb[:], y_ps[:], AF.Copy)
                dma.dma_start(io["y"][ot * 128:(ot + 1) * 128, pc * CW:(pc + 1) * CW],
                              y_sb[:])


def build_program():
    if "nc" in _CACHE:
        return _CACHE["nc"]
    nc = bacc.Bacc("TRN2", target_bir_lowering=False, debug=False, num_devices=8,
                   num_swdge_queues=4)
    io = {}
    io["xh"] = nc.dram_tensor("xh", (C, XW), F16, kind="ExternalInput").ap()
    io["xq"] = nc.dram_tensor("xq", (C, QXW), F16, kind="ExternalInput").ap()
    io["dv16"] = nc.dram_tensor("dv16", (128, 60 * 128), F16, kind="ExternalInput").ap()
    io["wpack"] = nc.dram_tensor("wpack", (C, 32), F32, kind="ExternalInput").ap()
    io["w2t"] = nc.dram_tensor("w2t", (C, 2), F32, kind="ExternalInput").ap()
    io["wot"] = nc.dram_tensor("wot", (C, C), F16, kind="ExternalInput").ap()
    io["refyx"] = nc.dram_tensor("refyx", (2, N), F32, kind="ExternalInput").ap()
    io["identF"] = nc.dram_tensor("identF", (128, 128), F32, kind="ExternalInput").ap()
    io["ident16"] = nc.dram_tensor("ident16", (128, 128), F16, kind="ExternalInput").ap()
    io["vtab"] = nc.dram_tensor("vtab", (PIXF, C), F16).ap()
    io["y"] = nc.dram_tensor("y", (C, HALF_PIX), F32, kind="ExternalOutput").ap()

    with tile.TileContext(nc) as tc:
        with contextlib.ExitStack() as ctx:
            _emit(nc, tc, ctx, io)
    nc.compile()
    _CACHE["nc"] = nc
    return nc




def host_prep(inputs):
    """Build the 8 per-core input maps from full inputs."""
    x = np.asarray(inputs["x"], np.float32)          # (B, C, H, W)
    xpad = np.zeros((B, C, XW), np.float32)
    xpad_img = np.pad(x, ((0, 0), (0, 0), (1, 1), (1, 1)))  # (B, C, 58, 58)
    xpad[:, :, :HP * HP] = xpad_img.reshape(B, C, HP * HP)

    wv = np.asarray(inputs["w_v"], np.float32).reshape(C, 9)
    wk = np.asarray(inputs["w_k"], np.float32).reshape(C, 9)
    bv = np.asarray(inputs["b_v"], np.float32)
    bk = np.asarray(inputs["b_k"], np.float32)
    dv16 = np.zeros((128, 60 * 128), np.float16)
    idx = np.arange(128)
    # blocks: 0-26 value taps, 27-53 key taps, 54-56 per-ct bv diag,
    # 57-59 per-ct bk diag (biases applied as a 10th matmul against ones)
    for ct in range(CT):
        for t in range(9):
            dv16[idx, (ct * 9 + t) * 128 + idx] = wv[ct * 128 + idx, t]
            dv16[idx, (27 + ct * 9 + t) * 128 + idx] = wk[ct * 128 + idx, t]
        dv16[idx, (54 + ct) * 128 + idx] = bv[ct * 128:(ct + 1) * 128]
        dv16[idx, (57 + ct) * 128 + idx] = bk[ct * 128:(ct + 1) * 128]

    wpack = np.zeros((C, 32), np.float32)
    wpack[:, 0:9] = np.asarray(inputs["w_q"], np.float32).reshape(C, 9)
    wpack[:, 18:27] = np.asarray(inputs["w_off1"], np.float32).reshape(C, 9)
    wpack[:, 27] = np.asarray(inputs["b_q"], np.float32)
    wpack[:, 29] = np.asarray(inputs["b_off1"], np.float32)
    wpack[:, 30] = np.asarray(inputs["ln_g"], np.float32)
    wpack[:, 31] = np.asarray(inputs["ln_b"], np.float32)

    shared = {
        "dv16": dv16,
        "wpack": wpack,
        "w2t": np.ascontiguousarray(np.asarray(inputs["w_off2"], np.float32).T),
        "wot": np.ascontiguousarray(np.asarray(inputs["w_out"], np.float32).T).astype(np.float16),
        "identF": np.eye(128, dtype=np.float32),
        "ident16": np.eye(128, dtype=np.float16),
    }
    ry = (np.arange(KH, dtype=np.float32) + 0.5) / KH * 2 - 1
    rx = (np.arange(KW, dtype=np.float32) + 0.5) / KW * 2 - 1
    shared["refyx"] = np.ascontiguousarray(
        np.stack([np.repeat(ry, KW), np.tile(rx, KH)]), dtype=np.float32)

    in_maps = []
    xh16 = [np.ascontiguousarray(xpad[b]).astype(np.float16) for b in range(B)]
    for core in range(8):
        b, half = core // 2, core % 2
        m = dict(shared)
        m["xh"] = xh16[b]
        r0 = half * HALF_ROWS
        xqs = np.zeros((C, QXW), np.float16)
        xqs[:, :QXW - 2] = xh16[b][:, r0 * HP:r0 * HP + QXW - 2]
        m["xq"] = xqs
        in_maps.append(m)
    return in_maps


def assemble(results):
    y = np.empty((B, C, H, W), np.float32)
    for core in range(8):
        b, half = core // 2, core % 2
        part = results[core]["y"].reshape(C, HALF_ROWS, W)
        y[b, :, half * HALF_ROWS:(half + 1) * HALF_ROWS, :] = part
    return y


def run(inputs, trace=False):
    nc = build_program()
    in_maps = host_prep(inputs)
    res = run_bass_kernel_spmd(nc, in_maps, core_ids=list(range(8)), trace=trace)
    return assemble(res.results), res


def kernel(**inputs):
    out, _ = run(inputs, trace=False)
    return out

